# revision 1
# baseline (speedup 1.0000x reference)
"""DBRX MoE experts kernel for Trainium2 (8 NeuronCores, expert-parallel).

Strategy:
  - Host: router (softmax top-2 + renorm), token dispatch (gather tokens per
    expert), weight/activation layout packing (transpose + bf16 cast).
  - Device (SPMD, 1 expert per core): for its expert's tokens X [C, D]:
        h = silu(X @ w1.T) * (X @ v1.T)   (I = 4096 intermediate)
        y = h @ w2.T                      ([C, D], pre-gate)
    bf16 matmuls, fp32 PSUM accumulation.
  - Host: combine: out[t] = sum_e gate[t,e] * y_e[t].

Device data layouts (per core / expert e), all partition-major so every DMA is
a contiguous [128, F] block:
  xt  [nD, 128, C]    bf16: xt[d, p, t]      = x_pad[t, 128 d + p]
  w1t [nI, 128, D]    bf16: w1t[it, p, f]    = w1[e][128 it + (f % 128), ...]
                        with f = 128*dchunk + m: w1t[it,p,f] = w1[e][128 it + m, 128 dchunk + p]
  v1t same as w1t
  w2t [nD, 128, I]    bf16: w2t[dt, p, f], f = 128*ichunk + m:
                        w2t[dt,p,f] = w2[e][128 dt + m, 128 ichunk + p]
  y   [nD, 128, C]    f32:  y[dt, p, t]      = y_e[t, 128 dt + p]
"""

import numpy as np

import concourse.bass as bass
from concourse import bacc, mybir, tile
from concourse.bass_utils import run_bass_kernel_spmd

BF16 = mybir.dt.bfloat16
F32 = mybir.dt.float32
NP_BF16 = mybir.dt.np(BF16)

P = 128  # partitions
NCHUNK = 512  # max moving free dim per matmul (one fp32 PSUM bank)


def _c_chunks(C):
    """Split token dim C into PSUM-bank-sized chunks."""
    out = []
    o = 0
    while o < C:
        s = min(NCHUNK, C - o)
        out.append((o, s))
        o += s
    return out


def build_nc(C, D, I, num_devices=8, iters=1, dma_spread=True):
    """Build the SPMD device program for one expert with C padded tokens.

    iters > 1 repeats the whole body (for slope-based HW timing).
    dma_spread: issue w2 prefetch on gpsimd and y stores on scalar so they
    don't queue behind the w1/v1 stream (and its waits) on the sync engine."""
    nD = D // P
    nI = I // P
    chunks = _c_chunks(C)

    nc = bacc.Bacc(
        "TRN2", target_bir_lowering=False, debug=False, num_devices=num_devices
    )
    xt_d = nc.dram_tensor("xt", [nD, P, C], BF16, kind="ExternalInput").ap()
    w1_d = nc.dram_tensor("w1t", [nI, P, D], BF16, kind="ExternalInput").ap()
    v1_d = nc.dram_tensor("v1t", [nI, P, D], BF16, kind="ExternalInput").ap()
    w2_d = nc.dram_tensor("w2t", [nD, P, I], BF16, kind="ExternalInput").ap()
    y_d = nc.dram_tensor("y", [nD, P, C], F32, kind="ExternalOutput").ap()

    with tile.TileContext(nc) as tc:
        with (
            tc.tile_pool(name="xres", bufs=1) as xres,
            tc.tile_pool(name="h2res", bufs=1) as h2res,
            tc.tile_pool(name="wload", bufs=6) as wload,
            tc.tile_pool(name="w2load", bufs=3) as w2load,
            tc.tile_pool(name="yout", bufs=2) as yout,
            tc.tile_pool(name="sgp", bufs=3) as sgp,
            tc.tile_pool(name="ps", bufs=8, space="PSUM") as ps,
        ):
          xt_sb = xres.tile([P, nD, C], BF16)
          for d in range(nD):
              nc.sync.dma_start(xt_sb[:, d, :], xt_d[d])
          for _rep in range(iters):
            h2_sb = h2res.tile([P, nI, C], BF16)

            # Phase 1: h2 = silu(x@w1.T) * (x@v1.T), laid out [I_part, C]
            for it in range(nI):
                w1sb = wload.tile([P, nD, P], BF16, tag="w")
                v1sb = wload.tile([P, nD, P], BF16, tag="w")
                nc.sync.dma_start(w1sb[:], w1_d[it])
                nc.sync.dma_start(v1sb[:], v1_d[it])
                for co, cs in chunks:
                    ph = ps.tile([P, NCHUNK], F32, tag="pp")
                    pg = ps.tile([P, NCHUNK], F32, tag="pp")
                    for d in range(nD):
                        nc.tensor.matmul(
                            ph[:, :cs],
                            w1sb[:, d, :],
                            xt_sb[:, d, co : co + cs],
                            start=(d == 0),
                            stop=(d == nD - 1),
                        )
                    for d in range(nD):
                        nc.tensor.matmul(
                            pg[:, :cs],
                            v1sb[:, d, :],
                            xt_sb[:, d, co : co + cs],
                            start=(d == 0),
                            stop=(d == nD - 1),
                        )
                    # silu(h)*g with <=1 PSUM operand per DVE instruction
                    sg = sgp.tile([P, NCHUNK], F32, tag="sg")
                    t1 = sgp.tile([P, NCHUNK], F32, tag="t1")
                    nc.scalar.activation(
                        sg[:, :cs], ph[:, :cs], mybir.ActivationFunctionType.Sigmoid
                    )
                    nc.vector.tensor_mul(t1[:, :cs], sg[:, :cs], ph[:, :cs])
                    nc.vector.tensor_mul(
                        h2_sb[:, it, co : co + cs], t1[:, :cs], pg[:, :cs]
                    )

            # Phase 2: y = h2.T @ w2.T, laid out [D_part, C]
            for dt in range(nD):
                w2sb = w2load.tile([P, nI, P], BF16, tag="w2")
                # gpsimd queue: prefetches during phase 1 instead of queueing
                # behind the w1/v1 stream (and its waits) on the sync engine
                (nc.gpsimd if dma_spread else nc.sync).dma_start(w2sb[:], w2_d[dt])
                ysb = yout.tile([P, C], F32)
                if True:
                    for co, cs in chunks:
                        py = ps.tile([P, NCHUNK], F32, tag="pp")
                        for ic in range(nI):
                            nc.tensor.matmul(
                                py[:, :cs],
                                w2sb[:, ic, :],
                                h2_sb[:, ic, co : co + cs],
                                start=(ic == 0),
                                stop=(ic == nI - 1),
                            )
                        nc.vector.tensor_copy(ysb[:, co : co + cs], py[:, :cs])
                # scalar (ACT) HWDGE queue: output stores must not stall
                # weight prefetch on the sync queue; ACT is idle in phase 2
                (nc.scalar if dma_spread else nc.sync).dma_start(y_d[dt], ysb[:])

    nc.compile()
    return nc


def pack_x(x_pad, nD):
    """[C, D] f32 -> [nD, 128, C] bf16."""
    C = x_pad.shape[0]
    return np.ascontiguousarray(x_pad.T.reshape(nD, P, C)).astype(NP_BF16)


def pack_w_up(w):
    """w1/v1 [I, D] -> [nI, 128, D] bf16 (lhsT tiles for the up-projections)."""
    I, D = w.shape
    a = w.reshape(I // P, P, D // P, P)  # [it, m, dchunk, p]
    return np.ascontiguousarray(a.transpose(0, 3, 2, 1).reshape(I // P, P, D)).astype(
        NP_BF16
    )


def pack_w_down(w):
    """w2 [D, I] -> [nD, 128, I] bf16 (lhsT tiles for the down-projection)."""
    D, I = w.shape
    a = w.reshape(D // P, P, I // P, P)  # [dt, m, ichunk, p]
    return np.ascontiguousarray(a.transpose(0, 3, 2, 1).reshape(D // P, P, I)).astype(
        NP_BF16
    )


def unpack_y(y, C):
    """[nD, 128, C] f32 -> [C, D] f32."""
    return y.transpose(2, 0, 1).reshape(C, -1)


def route(x, wr, top_k=2):
    """Softmax top-k with renormalization. Returns topi [T,k], topw [T,k]."""
    logits = x @ wr.T
    logits -= logits.max(-1, keepdims=True)
    p = np.exp(logits, dtype=np.float32)
    p /= p.sum(-1, keepdims=True)
    topi = np.argpartition(-p, top_k - 1, axis=-1)[:, :top_k]
    topw = np.take_along_axis(p, topi, -1)
    topw = topw / topw.sum(-1, keepdims=True)
    return topi, topw


_NC_CACHE = {}


def kernel(hidden_states, wr, w1, v1, w2, index):
    x = np.asarray(hidden_states, dtype=np.float32)
    wr = np.asarray(wr, dtype=np.float32)
    w1 = np.asarray(w1, dtype=np.float32)
    v1 = np.asarray(v1, dtype=np.float32)
    w2 = np.asarray(w2, dtype=np.float32)
    T, D = x.shape
    E, I, _ = w1.shape

    topi, topw = route(x, wr)
    idx = [np.nonzero((topi == e).any(-1))[0] for e in range(E)]
    gates = np.zeros((T, E), np.float32)
    np.put_along_axis(gates, topi, topw, axis=-1)

    mx = max(len(ix) for ix in idx)
    C = max(P, ((mx + 7) // 8) * 8)

    key = (C, D, I, E)
    if key not in _NC_CACHE:
        _NC_CACHE[key] = build_nc(C, D, I, num_devices=E)
    nc = _NC_CACHE[key]

    in_maps = []
    for e in range(E):
        x_pad = np.zeros((C, D), np.float32)
        x_pad[: len(idx[e])] = x[idx[e]]
        in_maps.append(
            {
                "xt": pack_x(x_pad, D // P),
                "w1t": pack_w_up(w1[e]),
                "v1t": pack_w_up(v1[e]),
                "w2t": pack_w_down(w2[e]),
            }
        )

    res = run_bass_kernel_spmd(nc, in_maps, core_ids=list(range(E)))

    out = np.zeros((T, D), np.float32)
    for e in range(E):
        y_e = unpack_y(res.results[e]["y"], C)[: len(idx[e])]
        out[idx[e]] += gates[idx[e], e][:, None] * y_e
    return out



# revision 3
# speedup vs baseline: 2.1890x; 2.1890x over previous
"""DBRX MoE experts kernel for Trainium2 — tensor-parallel over the
intermediate dim I (8 NeuronCores).

Each core holds rows [512k, 512(k+1)) of every expert's w1/v1 and the
matching columns of w2, and processes ALL routed (token, expert) pairs:

    h_e = silu(x_e @ w1_loc[e].T) * (x_e @ v1_loc[e].T)   # [C_e, 512]
    y_e_partial = h_e @ w2_loc[e].T                       # [C_e, D]

The 8 partial y's (bf16) are summed on the host in fp32 and combined
with the router gates. Every core does exactly sum_e C_e / 8 ~ 1024
tokens' worth of matmul work regardless of routing balance — the PE
roofline drops from 6*Cmax*D*I (Cmax ~ 1088) to 6*1024*D*I per core.

Program shape: ALL experts' phase 1, then all phase 2 (h2 for every
expert fits SBUF: [128, 4, ~8200] bf16 = 66 KB/partition). DMA
descriptor count is minimized (HWDGE costs ~0.6 us per descriptor):
x = 1 descriptor/chunk, w1+v1 merged = 1/il-tile, w2 = 1/two dt tiles,
y = 1/(expert, dt).

Layouts (identical on every core; only the weight DATA differs):
  xtc [n_chunks, 128, nD, CSMAX] bf16 — routed tokens, partition-major
  wv  [E*4, 128, 2*D]            bf16 — w1|v1 lhsT tiles interleaved
  w2t [E*8, 128, 2*4*128]        bf16 — w2 lhsT tiles, dt-pairs
  y   [nD, 128, CTC]             bf16 — partial outputs, expert-major cols
"""

import numpy as np

import concourse.bass as bass
from concourse import bacc, mybir, tile
from concourse.bass_utils import run_bass_kernel_spmd

BF16 = mybir.dt.bfloat16
F32 = mybir.dt.float32
NP_BF16 = mybir.dt.np(BF16)

P = 128
NCHUNK = 512  # PSUM bank: 512 fp32 cols max per matmul
T, D, E, I, TOP_K = 4096, 2048, 8, 4096, 2
NCORE = 8
IL = I // NCORE  # 512 rows of I per core
NIL = IL // P  # 4 local I tiles
ND = D // P  # 16 D tiles


def _balanced_chunks(C, starter=0):
    """Near-equal multiples of 8, each <= NCHUNK. Returns [(off, size)]."""
    out = []
    o = 0
    if starter and C > starter:
        out.append((0, starter))
        o = starter
        C -= starter
    if C <= 0:
        return out
    n = (C + NCHUNK - 1) // NCHUNK
    base = (C // n) // 8 * 8
    sizes = [base] * n
    rem = C - base * n
    i = 0
    while rem > 0:
        sizes[i] += 8
        rem -= 8
        i = (i + 1) % n
    for s in sizes:
        out.append((o, s))
        o += s
    return out


def make_plan(counts):
    """Balanced chunks; expert 0 gets a ramped plan (small chunks first)
    so PE work starts as soon as ~1.5 MB of DMA has landed and grows with
    the supply, instead of waiting for a full 512-col chunk + weights."""
    ce = [(c + 7) // 8 * 8 for c in counts]
    plan = []
    for e, c in enumerate(ce):
        if e == 0 and c > 768:
            ramp = [128, 256]
            rest = _balanced_chunks(c - 384)
            plan.append(
                [(0, 128), (128, 256)] + [(o + 384, s) for o, s in rest]
            )
        else:
            plan.append(_balanced_chunks(c))
    return plan


def build_nc(chunk_plan, csmax, num_devices=NCORE, iters=1):
    n_chunks = sum(len(ch) for ch in chunk_plan)
    gco, ge = [], []  # per-chunk / per-expert global col offsets
    o = 0
    for ch in chunk_plan:
        ge.append(o)
        for _, cs in ch:
            gco.append(o)
            o += cs
    ctc = o
    cemax = max(
        (ch[-1][0] + ch[-1][1] for ch in chunk_plan if ch), default=8
    )

    nc = bacc.Bacc(
        "TRN2", target_bir_lowering=False, debug=False, num_devices=num_devices
    )
    xtc_d = nc.dram_tensor("xtc", [n_chunks, P, ND, csmax], BF16, kind="ExternalInput").ap()
    wv_d = nc.dram_tensor("wv", [E * NIL, P, 2 * D], BF16, kind="ExternalInput").ap()
    w2_d = nc.dram_tensor("w2t", [E * ND // 2, P, 2 * NIL * P], BF16, kind="ExternalInput").ap()
    y_d = nc.dram_tensor("y", [ND, P, ctc], BF16, kind="ExternalOutput").ap()

    with tile.TileContext(nc) as tc:
        with (
            tc.tile_pool(name="xp", bufs=4) as xp,
            tc.tile_pool(name="h2p", bufs=1) as h2p,
            tc.tile_pool(name="wload", bufs=3) as wload,
            tc.tile_pool(name="w2load", bufs=3) as w2load,
            tc.tile_pool(name="yout", bufs=3) as yout,
            tc.tile_pool(name="sgp", bufs=4) as sgp,
            tc.tile_pool(name="ps", bufs=8, space="PSUM") as ps,
        ):
          # PE warmup: ~2.5 us of dummy matmuls on a memset tile while the
          # first x/weight DMAs are in flight. Costs nothing (PE would be
          # idle) and keeps the HAM clock-gate at full rate for the real
          # matmul stream (HW effect; the cost-model sim doesn't know HAM).
          wu = sgp.tile([P, NCHUNK], F32, tag="sg")
          nc.vector.memset(wu[:, :P], 0.0)
          wub = wu[:, :P].bitcast(BF16)[:, :P]
          wup = ps.tile([P, NCHUNK], F32, tag="pp")
          for i in range(24):
              nc.tensor.matmul(
                  wup[:, :P], wub, wub, start=(i == 0), stop=(i == 23)
              )
          for _rep in range(iters):
            h2 = h2p.tile([P, NIL, ctc], BF16)
            # ---------------- Phase 1 (all experts) ----------------
            cid = 0
            for e in range(E):
                chunks = chunk_plan[e]
                if not chunks:
                    continue
                cid_e = cid

                def emit_x(ci, cs):
                    xch = xp.tile([P, ND, csmax], BF16, tag="x")
                    nc.sync.dma_start(
                        xch[:, :, :cs], xtc_d[cid_e + ci][:, :, :cs]
                    )
                    return xch

                xchs = [emit_x(0, chunks[0][1])]
                for il in range(NIL):
                    wvsb = wload.tile([P, 2, ND, P], BF16, tag="w")
                    nc.sync.dma_start(wvsb[:], wv_d[e * NIL + il])
                    if il == 0:
                        for ci in range(1, len(chunks)):
                            xchs.append(emit_x(ci, chunks[ci][1]))
                    for ci, (co, cs) in enumerate(chunks):
                        xch = xchs[ci]
                        g = gco[cid + ci]
                        ph = ps.tile([P, NCHUNK], F32, tag="pp")
                        pg = ps.tile([P, NCHUNK], F32, tag="pp")
                        for d in range(ND):
                            nc.tensor.matmul(
                                ph[:, :cs],
                                wvsb[:, 0, d, :],
                                xch[:, d, :cs],
                                start=(d == 0),
                                stop=(d == ND - 1),
                            )
                        for d in range(ND):
                            nc.tensor.matmul(
                                pg[:, :cs],
                                wvsb[:, 1, d, :],
                                xch[:, d, :cs],
                                start=(d == 0),
                                stop=(d == ND - 1),
                            )
                        sg = sgp.tile([P, NCHUNK], F32, tag="sg")
                        t1 = sgp.tile([P, NCHUNK], F32, tag="t1")
                        nc.scalar.activation(
                            sg[:, :cs], ph[:, :cs],
                            mybir.ActivationFunctionType.Sigmoid,
                        )
                        nc.vector.tensor_mul(t1[:, :cs], sg[:, :cs], ph[:, :cs])
                        nc.vector.tensor_mul(
                            h2[:, il, g : g + cs], t1[:, :cs], pg[:, :cs]
                        )
                cid += len(chunks)

            # ---------------- Phase 2 (all experts) ----------------
            cid = 0
            flip = 0
            for e in range(E):
                chunks = chunk_plan[e]
                if not chunks:
                    continue
                ce_len = chunks[-1][0] + chunks[-1][1]
                for dt2 in range(ND // 2):
                    w2sb = w2load.tile([P, 2, NIL, P], BF16, tag="w2")
                    nc.scalar.dma_start(w2sb[:], w2_d[e * (ND // 2) + dt2])
                    for half in range(2):
                        dt = 2 * dt2 + half
                        ysb = yout.tile([P, cemax], BF16)
                        for ci, (co, cs) in enumerate(chunks):
                            g = gco[cid + ci]
                            py = ps.tile([P, NCHUNK], F32, tag="pp")
                            for ic in range(NIL):
                                nc.tensor.matmul(
                                    py[:, :cs],
                                    w2sb[:, half, ic, :],
                                    h2[:, ic, g : g + cs],
                                    start=(ic == 0),
                                    stop=(ic == NIL - 1),
                                )
                            nc.vector.tensor_copy(ysb[:, co : co + cs], py[:, :cs])
                        q = nc.sync if flip else nc.scalar
                        flip ^= 1
                        q.dma_start(
                            y_d[dt][:, ge[e] : ge[e] + ce_len], ysb[:, :ce_len]
                        )
                cid += len(chunks)

    nc.compile()
    return nc


def pack_w_up(w):
    """[Iloc, D] -> [Iloc//128, 128, D] bf16 lhsT tiles."""
    Il, Dd = w.shape
    a = w.reshape(Il // P, P, Dd // P, P)
    return np.ascontiguousarray(a.transpose(0, 3, 2, 1).reshape(Il // P, P, Dd)).astype(
        NP_BF16
    )


def pack_w_down(w):
    """[D, Iloc] -> [D//128, 128, Iloc] bf16 lhsT tiles."""
    Dd, Il = w.shape
    a = w.reshape(Dd // P, P, Il // P, P)
    return np.ascontiguousarray(a.transpose(0, 3, 2, 1).reshape(Dd // P, P, Il)).astype(
        NP_BF16
    )


def route(x, wr, top_k=TOP_K):
    logits = x @ wr.T
    logits -= logits.max(-1, keepdims=True)
    p = np.exp(logits, dtype=np.float32)
    p /= p.sum(-1, keepdims=True)
    topi = np.argpartition(-p, top_k - 1, axis=-1)[:, :top_k]
    topw = np.take_along_axis(p, topi, -1)
    topw = topw / topw.sum(-1, keepdims=True)
    return topi, topw


_NC_CACHE = {}


def prepare(hidden_states, wr, w1, v1, w2):
    """Route + pack everything. Returns (plan, csmax, in_maps, combine_ctx)."""
    x = np.asarray(hidden_states, dtype=np.float32)
    wr = np.asarray(wr, dtype=np.float32)
    w1 = np.asarray(w1, dtype=np.float32)
    v1 = np.asarray(v1, dtype=np.float32)
    w2 = np.asarray(w2, dtype=np.float32)

    topi, topw = route(x, wr)
    idx = [np.nonzero((topi == e).any(-1))[0] for e in range(E)]
    gates = np.zeros((T, E), np.float32)
    np.put_along_axis(gates, topi, topw, axis=-1)

    counts = [len(ix) for ix in idx]
    chunk_plan = make_plan(counts)
    csmax = max((s for ch in chunk_plan for _, s in ch), default=8)
    n_chunks = sum(len(ch) for ch in chunk_plan)

    # Pack x chunk-major, partition-major within a chunk.
    xtc = np.zeros((n_chunks, P, ND, csmax), NP_BF16)
    cid = 0
    chunk_owner = []  # (expert, local_off, size)
    ge = []
    o = 0
    for e in range(E):
        ge.append(o)
        xe = x[idx[e]]
        for co, cs in chunk_plan[e]:
            blk = np.zeros((cs, D), np.float32)
            val = xe[co : min(co + cs, counts[e])]
            blk[: len(val)] = val
            # [cs, D] -> [P, nD, cs]
            xtc[cid, :, :, :cs] = blk.T.reshape(ND, P, cs).transpose(1, 0, 2)
            chunk_owner.append((e, co, cs))
            cid += 1
            o += cs

    in_maps = []
    for k in range(NCORE):
        rows = slice(k * IL, (k + 1) * IL)
        wv = np.empty((E * NIL, P, 2 * D), NP_BF16)
        for e in range(E):
            wv[e * NIL : (e + 1) * NIL, :, :D] = pack_w_up(w1[e][rows])
            wv[e * NIL : (e + 1) * NIL, :, D:] = pack_w_up(v1[e][rows])
        w2t = np.concatenate(
            [pack_w_down(w2[e][:, rows]) for e in range(E)], axis=0
        ).reshape(E * ND // 2, 2, P, IL)
        # [pair, half, P, IL] -> [pair, P, half, IL] flattened
        w2t = np.ascontiguousarray(w2t.transpose(0, 2, 1, 3)).reshape(
            E * ND // 2, P, 2 * IL
        )
        in_maps.append({"xtc": xtc, "wv": wv, "w2t": w2t})

    combine_ctx = (idx, gates, counts, chunk_owner, ge)
    return chunk_plan, csmax, in_maps, combine_ctx


def kernel(hidden_states, wr, w1, v1, w2, index):
    chunk_plan, csmax, in_maps, combine_ctx = prepare(
        hidden_states, wr, w1, v1, w2
    )
    idx, gates, counts, chunk_owner, ge = combine_ctx

    key = tuple(tuple(ch) for ch in chunk_plan)
    if key not in _NC_CACHE:
        _NC_CACHE[key] = build_nc(chunk_plan, csmax)
    nc = _NC_CACHE[key]

    res = run_bass_kernel_spmd(nc, in_maps, core_ids=list(range(NCORE)))

    ysum = res.results[0]["y"].astype(np.float32)
    for k in range(1, NCORE):
        ysum += res.results[k]["y"].astype(np.float32)
    # ysum: [ND, P, CTC]

    out = np.zeros((T, D), np.float32)
    for cid, (e, co, cs) in enumerate(chunk_owner):
        n_valid = min(cs, counts[e] - co)
        if n_valid <= 0:
            continue
        g = ge[e] + co
        blk = ysum[:, :, g : g + n_valid].transpose(2, 0, 1).reshape(n_valid, D)
        rows = idx[e][co : co + n_valid]
        out[rows] += gates[rows, e][:, None] * blk
    return out
